# revision 7
# baseline (speedup 1.0000x reference)
"""Trainium2 Bass kernel: 5x5 window median+variance denoise filter.

y = relu(x - noise_var/(var5x5(x)+1e-10) * (x - median5x5(x) + noise_bias))
with zero-padded 5x5 windows, unbiased variance (ddof=1).

Sharding: pure data parallel, B=16 images split 2-per-core across 8 cores.

v2 engine split (DVE was the single bottleneck at ~292us static busy):
  - host uploads z = (x - 0.5 + nb) as fp16 (pad value nb-0.5). The median
    network is shift-equivariant, so mids_z = mid + nb and the formula
    becomes pure subtracts; variance is shift-invariant (biases folded into
    ACT squares). fp16 halves DVE time (2x_1p mode, HW-verified 1150ns/op)
    and halves DMA.
  - DVE: only the 90-op median comparator network (min/max are
    DVE-only: the Pool engine rejects them at compile).
  - Pool/GPSIMD: all variance adds + formula sub/mult (HW-verified
    4374ns/op, dtype-agnostic; only add/sub/mult are supported there).
  - ACT: squares (center bias folded), ln/exp reciprocal with ln(24nv)
    folded, relu with +0.5-nb folded, all off the critical engines.

Median via a pruned comparator network with shared column sorts:
  sort5 over the 5 dy-shifted planes (9 CE, shared by 5 horizontal windows)
  T = odd-even merge of adjacent sorted columns (13 CE, shared by 2 windows)
  final rank-12 selection from T(x-2), T(x), S(x+2) (35 CE, single-sided
  min/max pruned) -- verified offline by exhaustive 0-1 principle.
"""
import math

import numpy as np

import concourse.bass as bass  # noqa: F401
import concourse.mybir as mybir
from concourse import bacc, tile
from concourse.bass_utils import run_bass_kernel_spmd

F32 = mybir.dt.float32
FP16 = mybir.dt.float16
ALU = mybir.AluOpType
ACTF = mybir.ActivationFunctionType

# (i, j, need_min, need_max) per structure; designed + 0/1-verified offline.
SORT5 = [(0, 1, 1, 1), (3, 4, 1, 1), (2, 4, 1, 1), (2, 3, 1, 1), (0, 3, 1, 1),
         (0, 2, 1, 1), (1, 4, 1, 1), (1, 3, 1, 1), (1, 2, 1, 1)]
T_CES = [(0, 5, 1, 1), (4, 9, 1, 1), (4, 5, 1, 1), (2, 7, 1, 1), (2, 4, 1, 1),
         (7, 5, 1, 1), (1, 6, 1, 1), (3, 8, 1, 1), (3, 6, 1, 1), (1, 2, 1, 1),
         (3, 4, 1, 1), (6, 7, 1, 1), (8, 5, 1, 1)]
F_CES = [(0, 10, 0, 1), (5, 15, 1, 0), (5, 10, 1, 1), (4, 14, 1, 1),
         (4, 5, 0, 1), (14, 10, 1, 0), (2, 12, 0, 1), (7, 17, 1, 0),
         (7, 12, 1, 1), (7, 5, 0, 1), (12, 14, 1, 1), (1, 11, 0, 1),
         (9, 19, 1, 0), (9, 11, 1, 1), (6, 16, 1, 1), (6, 9, 0, 1),
         (16, 11, 1, 0), (3, 13, 0, 1), (8, 18, 1, 0), (8, 13, 1, 1),
         (8, 9, 1, 1), (13, 16, 1, 0), (8, 5, 1, 1), (9, 12, 1, 1),
         (13, 14, 1, 1), (8, 20, 0, 1), (13, 24, 1, 0), (13, 20, 0, 1),
         (9, 22, 0, 1), (22, 20, 1, 0), (5, 21, 0, 1), (14, 21, 1, 0),
         (12, 23, 1, 0), (12, 14, 0, 1), (14, 22, 1, 0)]
F_OUT = 14

H = 512
W = 512
IMGS_PER_CORE = 2
N_CORES = 8
WIDE = W + 4          # 2-col halo each side
BLK = 4               # row-blocks per chunk (one whole 512-row image)


class BufPool:
    """Free-list over preallocated fixed SBUF tensors. Tile's dependency
    tracker makes reuse safe (WAR/RAW serialization on the same tensor)."""

    def __init__(self, nc, tag="wb", dtype=F32, cap=40):
        self.nc = nc
        self.tag = tag
        self.dtype = dtype
        self.cap = cap
        self.bufs = []
        self.free = []

    def alloc(self):
        if self.free:
            return self.free.pop()
        idx = len(self.bufs)
        assert idx < self.cap, f"SBUF pool {self.tag} exhausted"
        t = self.nc.alloc_sbuf_tensor(f"{self.tag}{idx}", [128, BLK, WIDE],
                                      self.dtype).ap()
        self.bufs.append(t)
        return t

    def release(self, t):
        self.free.append(t)


class Wire:
    """SSA value living at column offset `off` of `buf`."""

    def __init__(self, buf, off, owned, pool, on_die=None):
        self.buf = buf
        self.off = off
        self.owned = owned      # release buf to pool when dead
        self.pool = pool
        self.reads_left = 0
        self.on_die = on_die

    def ap(self, width):
        return self.buf[:, :, self.off:self.off + width]

    def read_done(self):
        self.reads_left -= 1
        if self.reads_left == 0:
            self._die()

    def read_done_zero(self):
        if self.reads_left == 0:
            self._die()

    def _die(self):
        if self.owned:
            self.pool.release(self.buf)
        if self.on_die is not None:
            self.on_die()

    def detach_views(self, n_views):
        """Transfer buffer ownership to n_views future views; returns the
        on_die callback the views share. Call read_done() after to consume
        the terminal hold."""
        buf, owned, pool = self.buf, self.owned, self.pool
        self.owned = False
        state = {"n": n_views}

        def on_die():
            state["n"] -= 1
            if state["n"] == 0 and owned:
                pool.release(buf)
        return on_die


def run_stage(nc, pool, wires, ces, width, terminal_reads, fillers=None):
    """Emit one structure stage. A position's lifetime is split into segments
    at each rewrite; each Wire object gets the read count of its own segment
    so buffers release as soon as truly dead."""
    n = len(wires)
    # segs[i] = read counts per segment of position i (segment ends at a
    # write of i, which itself reads the old value).
    segs = [[] for _ in range(n)]
    cur = [0] * n
    for ci, (a, b, nmin, nmax) in enumerate(ces):
        cur[a] += 1
        cur[b] += 1
        if nmin:
            segs[a].append(cur[a])
            cur[a] = 0
        if nmax:
            segs[b].append(cur[b])
            cur[b] = 0
    for i in range(n):
        segs[i].append(cur[i] + terminal_reads.get(i, 0))

    seg_idx = [0] * n
    for i in range(n):
        wires[i].reads_left += segs[i][0]
        if segs[i][0] == 0:
            wires[i].read_done_zero()

    for ci, (i, j, nmin, nmax) in enumerate(ces):
        wi, wj = wires[i], wires[j]
        a = wi.ap(width)
        b = wj.ap(width)
        if nmin:
            lo = pool.alloc()
            nc.vector.tensor_tensor(lo[:, :, 0:width], a, b, ALU.min)
        if nmax:
            hi = pool.alloc()
            nc.vector.tensor_tensor(hi[:, :, 0:width], a, b, ALU.max)
        wi.read_done()
        wj.read_done()
        if nmin:
            seg_idx[i] += 1
            cnt = segs[i][seg_idx[i]]
            assert cnt > 0, "dead write (should be pruned offline)"
            wires[i] = Wire(lo, 0, True, pool)
            wires[i].reads_left = cnt
        if nmax:
            seg_idx[j] += 1
            cnt = segs[j][seg_idx[j]]
            assert cnt > 0, "dead write (should be pruned offline)"
            wires[j] = Wire(hi, 0, True, pool)
            wires[j].reads_left = cnt
        if fillers is not None:
            fl = next(fillers, None)
            if fl is not None:
                fl()


def emit_chunk(nc, pool, fpool, tin, out_tile, xa, ya, bias, img):
    """One image. tin: 5 dy-shifted fp16 tiles of z = x - 0.5 + nb.
    bias: dict of [128,1] f32 APs (mnb=-nb, m5nb=-5nb, ln24nv, relu=0.5-nb).
    """
    gp = nc.gpsimd
    full = lambda t: t[:, :, :]

    # ---- loads: 5 dy-shifted tiles [128, BLK, WIDE] from the padded shard
    for k in range(5):
        for b in range(BLK):
            s = img * (H + 4) + b * 128 + k
            nc.sync.dma_start(tin[k][:, b, :], xa[s: s + 128, :])

    # ---- variance, emitted as FILLER ops interleaved into the median
    # network's emission: the sort/merge stages are dependency-chains with
    # pipeline bubbles on DVE; these independent ops fill them. Each
    # filler() call emits one op. ----
    sq = [pool.alloc() for _ in range(2)]
    acc = pool.alloc()
    s01 = pool.alloc()
    s23 = pool.alloc()
    state = {}

    def mk_fillers():
        A = nc.scalar.activation
        V = nc.vector.tensor_tensor
        # q squares (ACT) + vertical adds, s vertical adds
        yield lambda: A(full(acc), full(tin[0]), ACTF.Square, bias["mnb"])
        yield lambda: A(full(sq[1]), full(tin[1]), ACTF.Square, bias["mnb"])
        yield lambda: V(full(acc), full(acc), full(sq[1]), ALU.add)
        yield lambda: A(full(sq[0]), full(tin[2]), ACTF.Square, bias["mnb"])
        yield lambda: V(full(acc), full(acc), full(sq[0]), ALU.add)
        yield lambda: A(full(sq[1]), full(tin[3]), ACTF.Square, bias["mnb"])
        yield lambda: V(full(acc), full(acc), full(sq[1]), ALU.add)
        yield lambda: A(full(sq[0]), full(tin[4]), ACTF.Square, bias["mnb"])
        yield lambda: V(full(acc), full(acc), full(sq[0]), ALU.add)
        yield lambda: V(full(s01), full(tin[0]), full(tin[1]), ALU.add)
        yield lambda: V(full(s23), full(tin[2]), full(tin[3]), ALU.add)
        yield lambda: V(full(s01), full(s01), full(s23), ALU.add)
        yield lambda: V(full(s23), full(s01), full(tin[4]), ALU.add)
        # s5 = s23; hsum s (tree into s01/t2/s01)
        yield lambda: V(s01[:, :, 0:W + 3], s23[:, :, 0:W + 3],
                        s23[:, :, 1:W + 4], ALU.add)
        t2 = pool.alloc()
        yield lambda: V(t2[:, :, 0:W + 1], s01[:, :, 0:W + 1],
                        s01[:, :, 2:W + 3], ALU.add)
        yield lambda: V(s01[:, :, 0:W], t2[:, :, 0:W],
                        s23[:, :, 4:W + 4], ALU.add)
        pool.release(t2)
        # sq_s = (sum xs / 5)^2 reusing s23
        yield lambda: A(s23[:, :, 0:W], s01[:, :, 0:W], ACTF.Square,
                        bias["m5nb"], 0.2)
        # hsum q (tree into sq[1]/sq[0]/sq[1])
        yield lambda: V(sq[1][:, :, 0:W + 3], acc[:, :, 0:W + 3],
                        acc[:, :, 1:W + 4], ALU.add)
        yield lambda: V(sq[0][:, :, 0:W + 1], sq[1][:, :, 0:W + 1],
                        sq[1][:, :, 2:W + 3], ALU.add)
        yield lambda: V(sq[1][:, :, 0:W], sq[0][:, :, 0:W],
                        acc[:, :, 4:W + 4], ALU.add)
        # d = q25 - sq_s (into sq[1]); rcp = exp(-ln(d+eps)+ln24nv)
        yield lambda: V(sq[1][:, :, 0:W], sq[1][:, :, 0:W],
                        s23[:, :, 0:W], ALU.subtract)
        ln = fpool.alloc()
        yield lambda: A(ln[:, :, 0:W], sq[1][:, :, 0:W], ACTF.Ln,
                        bias["eps"])
        rcp = pool.alloc()
        state["rcp"] = rcp
        yield lambda: A(rcp[:, :, 0:W], ln[:, :, 0:W], ACTF.Exp,
                        bias["ln24nv"], -1.0)
        fpool.release(ln)
        # early formula half: a2 = z*(1-rcp) - rcp*nb (the parts that do
        # not need the median; leaves a 2-op DVE tail after the last CE)
        one_m = sq[0]
        yield lambda: A(one_m[:, :, 0:W], rcp[:, :, 0:W], ACTF.Copy,
                        1.0, -1.0)
        a2 = pool.alloc()
        state["a2"] = a2
        zc_ = tin[2][:, :, 2:2 + W]
        yield lambda: V(a2[:, :, 0:W], zc_, one_m[:, :, 0:W], ALU.mult)
        rn = s23
        yield lambda: A(rn[:, :, 0:W], rcp[:, :, 0:W], ACTF.Copy,
                        0.0, bias["nb"])
        yield lambda: V(a2[:, :, 0:W], a2[:, :, 0:W], rn[:, :, 0:W],
                        ALU.subtract)
        pool.release(s01)
        pool.release(s23)
        pool.release(acc)
        pool.release(sq[0])
        pool.release(sq[1])

    fillers = mk_fillers()

    # ---- median network (all fp16, DVE only) ----
    s_wires = [Wire(tin[k], 0, False, pool) for k in range(5)]
    run_stage(nc, pool, s_wires, SORT5, WIDE, {k: 1 for k in range(5)},
              fillers=fillers)

    t_wires = [None] * 10
    c_views = [None] * 5
    for k in range(5):
        rk = s_wires[k]
        od = rk.detach_views(3)
        t_wires[k] = Wire(rk.buf, rk.off + 0, False, pool, on_die=od)
        t_wires[k + 5] = Wire(rk.buf, rk.off + 1, False, pool, on_die=od)
        c_views[k] = Wire(rk.buf, rk.off + 4, False, pool, on_die=od)
        rk.read_done()      # consume terminal hold

    run_stage(nc, pool, t_wires, T_CES, W + 3, {j: 1 for j in range(10)},
              fillers=fillers)

    f_wires = [None] * 25
    for j in range(10):
        tw = t_wires[j]
        od = tw.detach_views(2)
        f_wires[j] = Wire(tw.buf, tw.off + 0, False, pool, on_die=od)
        f_wires[j + 10] = Wire(tw.buf, tw.off + 2, False, pool, on_die=od)
        tw.read_done()
    for k in range(5):
        f_wires[20 + k] = c_views[k]

    run_stage(nc, pool, f_wires, F_CES, W, {F_OUT: 1}, fillers=fillers)
    for fl in fillers:       # drain any leftover fillers
        fl()
    mid = f_wires[F_OUT]     # median(z) = median(x) - 0.5 + nb

    # ---- formula tail (only 2 DVE ops after the last median CE):
    # y' = z(1-rcp) - rcp*nb + rcp*mid = a2 + rcp*mid ----
    rcp = state["rcp"]
    a2 = state["a2"]
    w2 = pool.alloc()
    nc.vector.tensor_tensor(w2[:, :, 0:W], rcp[:, :, 0:W], mid.ap(W),
                            ALU.mult)
    mid.read_done()
    pool.release(rcp)
    nc.vector.tensor_tensor(w2[:, :, 0:W], a2[:, :, 0:W], w2[:, :, 0:W],
                            ALU.add)
    pool.release(a2)
    nc.scalar.activation(out_tile[:, :, :], w2[:, :, 0:W], ACTF.Relu,
                         bias["relu"])
    pool.release(w2)

    # ---- store ----
    for b in range(BLK):
        nc.sync.dma_start(
            ya[img * H + b * 128: img * H + b * 128 + 128, :],
            out_tile[:, b, :],
        )


def build_module(repeat=1, hw_loop=None):
    nc = bacc.Bacc(
        "TRN2",
        target_bir_lowering=False,
        debug=False,
        enable_asserts=False,
        num_devices=N_CORES,
    )
    x = nc.dram_tensor("x", [IMGS_PER_CORE, H + 4, WIDE], FP16,
                       kind="ExternalInput")
    nvb = nc.dram_tensor("nvb", [128, 6], F32, kind="ExternalInput")
    y = nc.dram_tensor("y", [IMGS_PER_CORE, H, W], FP16,
                       kind="ExternalOutput")

    xa = x.ap().flatten_outer_dims()    # [2*516, 516]
    ya = y.ap().flatten_outer_dims()

    with tile.TileContext(nc) as tc:
        pool = BufPool(nc, "wh", FP16, 40)
        fpool = BufPool(nc, "wf", F32, 6)
        nvb_t = nc.alloc_sbuf_tensor("nvb_t", [128, 6], F32).ap()
        nc.sync.dma_start(nvb_t[:, :], nvb.ap()[:, :])
        bias = {
            "mnb": nvb_t[:, 0:1],
            "m5nb": nvb_t[:, 1:2],
            "ln24nv": nvb_t[:, 2:3],
            "relu": nvb_t[:, 3:4],
            "eps": nvb_t[:, 4:5],
            "nb": nvb_t[:, 5:6],
        }

        # double-buffered input tiles so chunk c+1's loads overlap compute
        tins = [[nc.alloc_sbuf_tensor(f"tin{p}_{k}", [128, BLK, WIDE],
                                      FP16).ap() for k in range(5)]
                for p in range(2)]
        out_t = [nc.alloc_sbuf_tensor(f"out{k}", [128, BLK, W], FP16).ap()
                 for k in range(2)]

        def body():
            ci = 0
            for _ in range(repeat):
                for img in range(IMGS_PER_CORE):
                    emit_chunk(nc, pool, fpool, tins[ci % 2],
                               out_t[ci % 2], xa, ya, bias, img)
                    ci += 1

        if hw_loop is None:
            body()
        else:
            with tc.For_i(0, hw_loop, 1):
                body()

    nc.compile()
    return nc


_MODULE = None


def _get_module():
    global _MODULE
    if _MODULE is None:
        _MODULE = build_module()
    return _MODULE


def prepare_inputs(x, nv, nb):
    """Host-side prep: z = (x - 0.5 + nb) fp16 padded to 516x516 with the
    zero-image value nb-0.5, plus the [128,4] bias table."""
    B = x.shape[0]
    nvb = np.empty((128, 6), np.float32)
    nvb[:, 0] = -nb
    nvb[:, 1] = -5.0 * nb
    nvb[:, 2] = math.log(24.0 * nv)
    nvb[:, 3] = 0.5 - nb
    nvb[:, 4] = 24e-10
    nvb[:, 5] = nb
    xpad = np.full((B, H + 4, WIDE), nb - 0.5, np.float32)
    xpad[:, 2:2 + H, 2:2 + W] = x[:, 0] + (nb - 0.5)
    xpad = xpad.astype(np.float16)
    in_maps = []
    for c in range(N_CORES):
        shard = np.ascontiguousarray(
            xpad[c * IMGS_PER_CORE:(c + 1) * IMGS_PER_CORE])
        in_maps.append({"x": shard, "nvb": nvb})
    return in_maps


def kernel(x, noise_var, noise_bias):
    x = np.ascontiguousarray(np.asarray(x, dtype=np.float32))
    nv = float(np.asarray(noise_var).reshape(-1)[0])
    nb = float(np.asarray(noise_bias).reshape(-1)[0])
    B = x.shape[0]
    assert x.shape == (B, 1, H, W) and B == N_CORES * IMGS_PER_CORE

    nc = _get_module()
    in_maps = prepare_inputs(x, nv, nb)
    res = run_bass_kernel_spmd(nc, in_maps, core_ids=list(range(N_CORES)))
    y = np.empty((B, 1, H, W), np.float32)
    for c in range(N_CORES):
        y[c * IMGS_PER_CORE:(c + 1) * IMGS_PER_CORE, 0] = \
            res.results[c]["y"].astype(np.float32)
    return y
